# revision 7
# baseline (speedup 1.0000x reference)
"""Trainium2 Bass kernel for nn_LstmModel_31851477467289.

Model: 3 chained GRUCells (enc -> {mask-dec, pred-dec}) over T=512 steps,
B=256, H=256, F=1, P=256.  mask = per-step linear of h_mask; pred = mean over
t of linear(h_pred).

Strategy (v0): pure data-parallel over batch across 8 cores (B_local=32).
Each core runs all three GRU chains for its batch slice — no cross-core
communication.  Within a step, matmuls keep the hidden state stationary
(lhsT = h^T chunks) and stream the (transposed) weight matrices, which is the
stream-efficient orientation for small batch.  Biases and the F=1 encoder
input are folded into the matmuls as extra K=1/K=2 rows.  pred-mean is
accumulated in PSUM across steps via an identity-stationary matmul.
"""
import os
import sys

sys.path.insert(0, "/opt/trn_rl_repo")

import numpy as np
from contextlib import ExitStack

import concourse.bass as bass
import concourse.mybir as mybir
import concourse.tile as tile
from concourse import bass_utils
from concourse.vector_clock import ScopedClock

F32 = mybir.dt.float32
F32R = mybir.dt.float32r
AF = mybir.ActivationFunctionType

T_FULL, B_FULL, F_IN, H, P = 512, 256, 1, 256, 256
N_CORES = 8
BL = B_FULL // N_CORES  # 32 per core

USE_F32R = True
MMDT = F32R if USE_F32R else F32

# ---------------------------------------------------------------------------
# Patch: this container's walrus build rejects Tile's kernel-tail Drain when
# it carries many semaphore waits ("Too many sync wait commands").  Split the
# tail waits across multiple sync-engine nops (1 wait each) before draining.
# ---------------------------------------------------------------------------
def _drain_and_barrier_split(self, tick_clock, wait_clock):
    nc = self.nc
    probe = nc.sync.nop(nofuse=True, hint="tail_wait_collect").ins
    wait_clock.add_sem_waits(probe, ScopedClock({None: tick_clock.global_clock}))
    waits = list(probe.sync_info.on_wait)
    probe.sync_info = mybir.SyncInfo(on_wait=[], on_update=[])
    for w in waits:
        inst = nc.sync.nop(nofuse=True, hint="tail_wait_split").ins
        inst.sync_info = mybir.SyncInfo(on_wait=[w], on_update=[])
    nc.sync.drain()

    nc.all_engine_barrier()
    assert self.sems is not None
    popped = nc._tile_sem_poison_stack.pop()
    assert popped is self._sem_poison
    nc.clear_and_free_semaphores(list(self.sems.allocated().values()))
    nc.all_engine_barrier()


tile.TileContext._drain_and_barrier = _drain_and_barrier_split


def _split_waits(nc, lim=1):
    """This walrus build allows very few sync-wait commands per instruction.
    Hoist excess waits onto preceding same-engine nops."""
    n = 0
    for f in nc.m.functions:
        for b in f.blocks:
            out = []
            for inst in b.instructions:
                si = inst.sync_info
                waits = list(si.on_wait) if si is not None else []
                if len(waits) > lim and inst.engine != mybir.EngineType.Unassigned:
                    head, keep = waits[:-lim], waits[-lim:]
                    for j, w in enumerate(head):
                        nop = mybir.InstNoOp(
                            name=f"{inst.name}_wsplit{j}", engine=inst.engine,
                            ins=[], outs=[], bass_nofuse=True)
                        nop.sync_info = mybir.SyncInfo(on_wait=[w], on_update=[])
                        out.append(nop)
                        n += 1
                    inst.sync_info = mybir.SyncInfo(
                        on_wait=keep, on_update=list(si.on_update))
                out.append(inst)
            b.instructions = out
    return n


# ---------------------------------------------------------------------------
# Device program
# ---------------------------------------------------------------------------
def build_nc(T, reps=1):
    nc = bass.Bass("TRN2", target_bir_lowering=False, debug=False,
                   enable_asserts=False, num_devices=N_CORES)

    # --- DRAM I/O (per core) ---
    d_xf = nc.dram_tensor("xf", [2, T * BL], MMDT, kind="ExternalInput")
    d_h0 = {c: nc.dram_tensor(f"h0_{c}", [BL, H], MMDT, kind="ExternalInput")
            for c in ("e", "m", "p")}
    d_h0T = {c: nc.dram_tensor(f"h0T_{c}", [H, BL], MMDT, kind="ExternalInput")
             for c in ("e", "m", "p")}
    d_enc_whhT = nc.dram_tensor("enc_whhT", [H, 3 * H], MMDT, kind="ExternalInput")
    d_mdec_wT = nc.dram_tensor("mdec_wT", [2 * H, 3 * H], MMDT, kind="ExternalInput")
    d_pdec_wT = nc.dram_tensor("pdec_wT", [2 * H, 3 * H], MMDT, kind="ExternalInput")
    d_plinT = nc.dram_tensor("plinT", [H, P], MMDT, kind="ExternalInput")
    d_mlinT = nc.dram_tensor("mlinT", [H, 1], MMDT, kind="ExternalInput")
    d_enc_fold = nc.dram_tensor("enc_fold", [2, 3 * H], MMDT, kind="ExternalInput")
    d_mdec_fold = nc.dram_tensor("mdec_fold", [1, 3 * H], MMDT, kind="ExternalInput")
    d_pdec_fold = nc.dram_tensor("pdec_fold", [1, 3 * H], MMDT, kind="ExternalInput")
    d_bhhn = {c: nc.dram_tensor(f"bhhn_{c}", [1, H], MMDT, kind="ExternalInput")
              for c in ("e", "m", "p")}
    d_ones = nc.dram_tensor("ones1", [1, BL], MMDT, kind="ExternalInput")
    d_eye = nc.dram_tensor("eye", [BL, BL], MMDT, kind="ExternalInput")
    d_linb = nc.dram_tensor("linb", [1, P + 1], MMDT, kind="ExternalInput")

    d_mask = nc.dram_tensor("mask_out", [BL, T], F32, kind="ExternalOutput")
    d_pred = nc.dram_tensor("pred_out", [BL, P], F32, kind="ExternalOutput")

    with tile.TileContext(nc) as tc:
        with ExitStack() as ctx:
            wpool = ctx.enter_context(tc.tile_pool(name="w", bufs=1))
            state = ctx.enter_context(tc.tile_pool(name="state", bufs=2))
            hTp = ctx.enter_context(tc.tile_pool(name="hT", bufs=2))
            gates = ctx.enter_context(tc.tile_pool(name="gates", bufs=2))
            ps_arz = ctx.enter_context(tc.tile_pool(name="ps_arz", bufs=2, space="PSUM"))
            ps_gin = ctx.enter_context(tc.tile_pool(name="ps_gin", bufs=2, space="PSUM"))
            ps_ghn = ctx.enter_context(tc.tile_pool(name="ps_ghn", bufs=2, space="PSUM"))
            ps_tr = ctx.enter_context(tc.tile_pool(name="ps_tr", bufs=1, space="PSUM"))
            ps_acc = ctx.enter_context(tc.tile_pool(name="ps_acc", bufs=1, space="PSUM"))

            def load(dram, shape, row0=0):
                t = wpool.tile(shape, MMDT, tag=dram.name + str(row0))
                nc.sync.dma_start(t[:], dram.ap()[row0:row0 + shape[0], :])
                return t

            # --- static SBUF residents ---
            xf = load(d_xf, [2, T * BL])
            enc_whhT = [load(d_enc_whhT, [128, 3 * H], 128 * k) for k in range(2)]
            mdec_wT = [load(d_mdec_wT, [128, 3 * H], 128 * k) for k in range(4)]
            pdec_wT = [load(d_pdec_wT, [128, 3 * H], 128 * k) for k in range(4)]
            plinT = [load(d_plinT, [128, P], 128 * k) for k in range(2)]
            mlinT = [load(d_mlinT, [128, 1], 128 * k) for k in range(2)]
            enc_fold = load(d_enc_fold, [2, 3 * H])
            mdec_fold = load(d_mdec_fold, [1, 3 * H])
            pdec_fold = load(d_pdec_fold, [1, 3 * H])
            bhhn = {c: load(d_bhhn[c], [1, H]) for c in ("e", "m", "p")}
            ones1 = load(d_ones, [1, BL])
            eye = load(d_eye, [BL, BL])
            linb = load(d_linb, [1, P + 1])

            mask_sb = wpool.tile([BL, T], F32, tag="mask_sb")
            pred_acc = ps_acc.tile([BL, P], F32, tag="pred_acc")

            def mm_group(psum_ap, mms):
                last = len(mms) - 1
                for i, (l, r) in enumerate(mms):
                    nc.tensor.matmul(psum_ap, l, r,
                                     start=(i == 0), stop=(i == last))

            def transpose_pair(h_new, tag):
                """h_new [BL, 256] -> two [128, BL] SBUF tiles (h_new^T)."""
                out = []
                for k in range(2):
                    ps = ps_tr.tile([128, BL], MMDT, tag="tr")
                    nc.tensor.transpose(ps[:], h_new[:, 128 * k:128 * (k + 1)], eye[:])
                    ht = hTp.tile([128, BL], MMDT, tag=f"hT_{tag}{k}")
                    if k == 0:
                        nc.scalar.copy(ht[:], ps[:])
                    else:
                        nc.vector.tensor_copy(ht[:], ps[:])
                    out.append(ht)
                return out

            def cell(tag, h_prev, arz_mms, gin_mms, ghn_mms):
                arz = ps_arz.tile([BL, 2 * H], F32, tag="arz")
                mm_group(arz[:], arz_mms)
                gin = ps_gin.tile([BL, H], F32, tag="gin")
                mm_group(gin[:], gin_mms)
                ghn = ps_ghn.tile([BL, H], F32, tag="ghn")
                mm_group(ghn[:], ghn_mms)

                rz = gates.tile([BL, 2 * H], F32, tag=f"rz_{tag}")
                nc.scalar.activation(rz[:], arz[:], AF.Sigmoid)
                t1 = gates.tile([BL, H], F32, tag=f"t1_{tag}")
                nc.vector.tensor_mul(t1[:], rz[:, 0:H], ghn[:])
                t2 = gates.tile([BL, H], F32, tag=f"t2_{tag}")
                nc.vector.tensor_add(t2[:], t1[:], gin[:])
                n = gates.tile([BL, H], F32, tag=f"n_{tag}")
                nc.scalar.activation(n[:], t2[:], AF.Tanh)
                d = gates.tile([BL, H], F32, tag=f"d_{tag}")
                nc.gpsimd.tensor_sub(d[:], h_prev[:], n[:])
                e = gates.tile([BL, H], F32, tag=f"e_{tag}")
                nc.vector.tensor_mul(e[:], rz[:, H:2 * H], d[:])
                h_new = state.tile([BL, H], MMDT, tag=f"h_{tag}")
                nc.gpsimd.tensor_add(h_new[:], n[:], e[:])
                return h_new

            for rep in range(reps):
                # --- initial state ---
                h_cur = {}
                hT_cur = {}
                for c in ("e", "m", "p"):
                    h0 = state.tile([BL, H], MMDT, tag=f"h_{c}")
                    nc.sync.dma_start(h0[:], d_h0[c].ap()[:, :])
                    h_cur[c] = h0
                    hts = []
                    for k in range(2):
                        ht = hTp.tile([128, BL], MMDT, tag=f"hT_{c}{k}")
                        nc.sync.dma_start(ht[:], d_h0T[c].ap()[128 * k:128 * (k + 1), :])
                        hts.append(ht)
                    hT_cur[c] = hts

                for t in range(T):
                    xf_t = xf[:, t * BL:(t + 1) * BL]
                    eT = hT_cur["e"]
                    # ---- enc ----
                    h_e = cell(
                        "e", h_cur["e"],
                        arz_mms=[(eT[0][:], enc_whhT[0][:, 0:2 * H]),
                                 (eT[1][:], enc_whhT[1][:, 0:2 * H]),
                                 (xf_t, enc_fold[:, 0:2 * H])],
                        gin_mms=[(xf_t, enc_fold[:, 2 * H:3 * H])],
                        ghn_mms=[(eT[0][:], enc_whhT[0][:, 2 * H:3 * H]),
                                 (eT[1][:], enc_whhT[1][:, 2 * H:3 * H]),
                                 (ones1[:], bhhn["e"][:])],
                    )
                    h_cur["e"] = h_e
                    eTn = transpose_pair(h_e, "e")
                    hT_cur["e"] = eTn

                    # ---- mask decoder ----
                    mT = hT_cur["m"]
                    h_m = cell(
                        "m", h_cur["m"],
                        arz_mms=[(eTn[0][:], mdec_wT[0][:, 0:2 * H]),
                                 (eTn[1][:], mdec_wT[1][:, 0:2 * H]),
                                 (mT[0][:], mdec_wT[2][:, 0:2 * H]),
                                 (mT[1][:], mdec_wT[3][:, 0:2 * H]),
                                 (ones1[:], mdec_fold[:, 0:2 * H])],
                        gin_mms=[(eTn[0][:], mdec_wT[0][:, 2 * H:3 * H]),
                                 (eTn[1][:], mdec_wT[1][:, 2 * H:3 * H]),
                                 (ones1[:], mdec_fold[:, 2 * H:3 * H])],
                        ghn_mms=[(mT[0][:], mdec_wT[2][:, 2 * H:3 * H]),
                                 (mT[1][:], mdec_wT[3][:, 2 * H:3 * H]),
                                 (ones1[:], bhhn["m"][:])],
                    )
                    h_cur["m"] = h_m
                    mTn = transpose_pair(h_m, "m")
                    hT_cur["m"] = mTn

                    # mask_t = h_m @ mlin^T + b
                    mps = ps_gin.tile([BL, 1], F32, tag="gin")
                    mm_group(mps[:], [(mTn[0][:].bitcast(F32), mlinT[0][:].bitcast(F32)),
                                      (mTn[1][:].bitcast(F32), mlinT[1][:].bitcast(F32)),
                                      (ones1[:].bitcast(F32), linb[:, P:P + 1].bitcast(F32))])
                    nc.scalar.copy(mask_sb[:, t:t + 1], mps[:])

                    # ---- pred decoder ----
                    pT = hT_cur["p"]
                    h_p = cell(
                        "p", h_cur["p"],
                        arz_mms=[(eTn[0][:], pdec_wT[0][:, 0:2 * H]),
                                 (eTn[1][:], pdec_wT[1][:, 0:2 * H]),
                                 (pT[0][:], pdec_wT[2][:, 0:2 * H]),
                                 (pT[1][:], pdec_wT[3][:, 0:2 * H]),
                                 (ones1[:], pdec_fold[:, 0:2 * H])],
                        gin_mms=[(eTn[0][:], pdec_wT[0][:, 2 * H:3 * H]),
                                 (eTn[1][:], pdec_wT[1][:, 2 * H:3 * H]),
                                 (ones1[:], pdec_fold[:, 2 * H:3 * H])],
                        ghn_mms=[(pT[0][:], pdec_wT[2][:, 2 * H:3 * H]),
                                 (pT[1][:], pdec_wT[3][:, 2 * H:3 * H]),
                                 (ones1[:], bhhn["p"][:])],
                    )
                    h_cur["p"] = h_p
                    pTn = transpose_pair(h_p, "p")
                    hT_cur["p"] = pTn

                    # pred_acc += h_p   (identity-stationary accumulate)
                    nc.tensor.matmul(pred_acc[:], eye[:], h_p[:],
                                     start=(t == 0), stop=(t == T - 1))

            # ---- final pred = (pred_acc/T) @ plin^T + plin_b ----
            pm = gates.tile([BL, H], MMDT, tag="pm")
            nc.scalar.activation(pm[:], pred_acc[:], AF.Copy, scale=1.0 / T)
            pmT = transpose_pair(pm, "pm")
            pred_ps = ps_arz.tile([BL, P], F32, tag="arz")
            mm_group(pred_ps[:], [(pmT[0][:], plinT[0][:]),
                                  (pmT[1][:], plinT[1][:]),
                                  (ones1[:], linb[:, 0:P])])
            pred_sb = gates.tile([BL, P], F32, tag="pred_sb")
            nc.scalar.copy(pred_sb[:], pred_ps[:])
            nc.sync.dma_start(d_pred.ap()[:, :], pred_sb[:])
            nc.sync.dma_start(d_mask.ap()[:, :], mask_sb[:])

    _split_waits(nc, lim=1)
    return nc


# ---------------------------------------------------------------------------
# Host side
# ---------------------------------------------------------------------------
_NC_CACHE = {}


def get_nc(T, reps=1):
    key = (T, reps)
    if key not in _NC_CACHE:
        _NC_CACHE[key] = build_nc(T, reps)
    return _NC_CACHE[key]


def make_in_maps(inputs, T):
    f32 = np.float32
    seq = np.asarray(inputs["sequence"], f32)[:T]          # [T, B, 1]
    pad = np.asarray(inputs["delay_pad"], f32)             # [1, B, 1]
    seq = np.concatenate([pad, seq[:-1]], axis=0)[:, :, 0]  # [T, B]

    def gw(n):
        return np.ascontiguousarray(np.asarray(inputs[n], f32))

    enc_whhT = np.ascontiguousarray(gw("enc_w_hh").T)              # [256, 768]
    mdec_wT = np.ascontiguousarray(
        np.concatenate([gw("mdec_w_ih"), gw("mdec_w_hh")], axis=1).T)  # [512, 768]
    pdec_wT = np.ascontiguousarray(
        np.concatenate([gw("pdec_w_ih"), gw("pdec_w_hh")], axis=1).T)
    plinT = np.ascontiguousarray(gw("plin_w").T)                   # [256, 256]
    mlinT = np.ascontiguousarray(gw("mlin_w").T)                   # [256, 1]

    def fold_rows(b_ih, b_hh, w_ih_row=None):
        # row covering gate columns: [b_ih+b_hh for r,z | b_ih for n]
        row = np.concatenate([(b_ih + b_hh)[:2 * H], b_ih[2 * H:]])
        if w_ih_row is None:
            return row[None, :]                                    # [1, 768]
        return np.stack([w_ih_row, row])                           # [2, 768]

    enc_fold = fold_rows(gw("enc_b_ih"), gw("enc_b_hh"), gw("enc_w_ih")[:, 0])
    mdec_fold = fold_rows(gw("mdec_b_ih"), gw("mdec_b_hh"))
    pdec_fold = fold_rows(gw("pdec_b_ih"), gw("pdec_b_hh"))
    bhhn = {"e": gw("enc_b_hh")[None, 2 * H:],
            "m": gw("mdec_b_hh")[None, 2 * H:],
            "p": gw("pdec_b_hh")[None, 2 * H:]}
    linb = np.concatenate([gw("plin_b"), gw("mlin_b")])[None, :]   # [1, P+1]

    shared = {
        "enc_whhT": enc_whhT, "mdec_wT": mdec_wT, "pdec_wT": pdec_wT,
        "plinT": plinT, "mlinT": mlinT,
        "enc_fold": enc_fold, "mdec_fold": mdec_fold, "pdec_fold": pdec_fold,
        "bhhn_e": bhhn["e"], "bhhn_m": bhhn["m"], "bhhn_p": bhhn["p"],
        "ones1": np.ones((1, BL), f32), "eye": np.eye(BL, dtype=f32),
        "linb": linb,
    }
    h0 = {"e": gw("h0_enc"), "m": gw("h0_mask"), "p": gw("h0_pred")}

    in_maps = []
    for c in range(N_CORES):
        sl = slice(c * BL, (c + 1) * BL)
        xf = np.empty((2, T * BL), f32)
        xf[0] = seq[:, sl].reshape(-1)
        xf[1] = 1.0
        m = dict(shared)
        m["xf"] = xf
        for k in ("e", "m", "p"):
            m[f"h0_{k}"] = np.ascontiguousarray(h0[k][sl])
            m[f"h0T_{k}"] = np.ascontiguousarray(h0[k][sl].T)
        in_maps.append(m)
    return in_maps


def run(inputs, T=T_FULL, reps=1):
    nc = get_nc(T, reps)
    in_maps = make_in_maps(inputs, T)
    res = bass_utils.run_bass_kernel_spmd(nc, in_maps, core_ids=list(range(N_CORES)))
    masks = [r["mask_out"] for r in res.results]          # each [BL, T]
    mask = np.concatenate(masks, axis=0)                  # [B, T]
    mask = np.ascontiguousarray(mask.T)[:, :, None]       # [T, B, 1]
    pred = np.concatenate([r["pred_out"] for r in res.results], axis=0)
    return mask.astype(np.float32), pred.astype(np.float32)


def kernel(**inputs):
    return run(inputs, T=T_FULL, reps=1)
